# revision 1
# baseline (speedup 1.0000x reference)
"""MoE HyperNet linear layer on 8 Trainium2 NeuronCores.

Reference computation (B=4096, I=O=1024, C=128, E=8):
    h      = relu(cond @ g_w1 + g_b1)                # [B, 4E]
    gating = softmax(h @ g_w2 + g_b2, axis=1)        # [B, E]
    out    = einsum('be,beo->bo', gating,
                    einsum('bi,eio->beo', x, W)) + gating @ expert_biases

Strategy: data-parallel shard B across the 8 cores (512 rows each),
replicate all weights, and fold the gate into the activations:

    out[b,o] = sum_e sum_i (g[b,e]*x[b,i]) W_e[i,o] + (gating @ biases)[b,o]

so the whole MoE collapses into ONE K=8192 GEMM per core that the PE
accumulates entirely in PSUM — no per-expert combine pass.

Per core:
  - x/cond shards are passed in pre-transposed ([feature, batch]) — a
    host-side layout choice during sharding, like the [E*I, O] W reshape.
  - gating MLP runs transposed ([4E,512] -> [8,512]); softmax over the 8
    experts via exp + an all-ones K=8 matmul + reciprocal (no max-shift:
    logits here are O(1)).
  - gate rows are broadcast to 128 partitions with one-hot selector
    matmuls (gb_all), then xtg_e = xT * g_e (DVE, output rounded to
    float32r) feeds the PE as the stationary operand.
  - main GEMM: out[bc][b,o] += xtg_e[ic,bc].T @ W_e[ic,oh] accumulated
    over all (e, ic) in 4 persistent [128,1024] PSUM tiles (8 banks);
    the expert-bias term (gT.T @ biases) is appended to the same
    accumulation chain before stop.
  - output is produced in natural [b, o] orientation; the host just
    concatenates core shards.

Big-GEMM operands are float32r (fast fp32 PE mode, ~1 cycle/row at
N>=256 vs 4 for plain fp32, rel.err ~1e-4): W/sel/eb are rounded by
casting gpsimd DMAs, xtg/gT by DVE output dtype.

Any instruction here can carry only ONE sync wait (walrus limit), so a
post-pass splits extra waits onto same-engine NoOps (_split_waits).
"""

import sys

if "/opt/trn_rl_repo" not in sys.path:
    sys.path.insert(0, "/opt/trn_rl_repo")

import numpy as np

import bass_rust
import concourse.bass as bass
import concourse.mybir as mybir
import concourse.tile as tile
from concourse.bass_utils import run_bass_kernel_spmd


def _split_waits(nc, max_waits=1):
    """Hoist all-but-one sync wait of each instruction onto same-engine
    NoOps inserted directly before it. This walrus build rejects any TPB
    instruction carrying more than one wait ("Too many sync wait
    commands"); engines are in-order so the split preserves semantics."""
    for bb in nc.m.functions[0].blocks:
        out = []
        for i in list(bb.instructions):
            si = i.sync_info
            waits = list(si.on_wait) if si else []
            if len(waits) > max_waits:
                for k, w in enumerate(waits[:-max_waits]):
                    nop = mybir.InstNoOp(
                        name=f"{i.name}-waitsplit{k}", ins=[], outs=[])
                    nop.engine = i.engine
                    nop.sync_info = bass_rust.SyncInfo(on_wait=[w], on_update=[])
                    out.append(nop)
                i.sync_info = bass_rust.SyncInfo(
                    on_wait=waits[-max_waits:], on_update=list(si.on_update))
            out.append(i)
        bb.instructions = out

B, I, O, C, E = 4096, 1024, 1024, 128, 8
N_CORES = 8
BS = B // N_CORES          # 512 batch rows per core
NB = BS // 128             # 4 batch chunks of 128
NI = I // 128              # 8 contraction chunks
NO2 = 2                    # two N=512 halves of O
H = 4 * E                  # 32 gating hidden

_cache = {}


def _build_nc():
    dt = mybir.dt
    f32, f32r = dt.float32, dt.float32r

    nc = bass.Bass("TRN2", target_bir_lowering=False, debug=False,
                   num_devices=N_CORES)

    xT_d = nc.dram_tensor("xT_sh", [I, BS], f32, kind="ExternalInput").ap()
    condT_d = nc.dram_tensor("condT_sh", [C, BS], f32, kind="ExternalInput").ap()
    w_d = nc.dram_tensor("w", [E * I, O], f32, kind="ExternalInput").ap()
    eb_d = nc.dram_tensor("eb", [E, O], f32, kind="ExternalInput").ap()
    gpack_d = nc.dram_tensor("gpack", [128, 50], f32, kind="ExternalInput").ap()
    sel_d = nc.dram_tensor("sel", [E, E * 128], f32, kind="ExternalInput").ap()
    out_d = nc.dram_tensor("out_sh", [BS, O], f32, kind="ExternalOutput").ap()

    with tile.TileContext(nc) as tc:
        with (
            tc.tile_pool(name="consts", bufs=1) as consts,
            tc.tile_pool(name="xin", bufs=2) as xin,
            tc.tile_pool(name="stage", bufs=1) as stage,
            tc.tile_pool(name="wpool", bufs=2) as wpool,
            tc.tile_pool(name="xtgp", bufs=2) as xtgp,
            tc.tile_pool(name="outp", bufs=2) as outp,
        ):
            # ---- constants: one packed DMA for the whole gating MLP ----
            gpack = consts.tile([128, 50], f32, tag="gpack")
            nc.scalar.dma_start(gpack[:], gpack_d)
            gw1 = gpack[:, 0:H]            # [128, 32]
            gb1 = gpack[0:H, H:H + 1]      # [32, 1]
            gw2 = gpack[0:H, 33:33 + E]    # [32, 8]
            gb2 = gpack[0:E, 41:42]        # [8, 1]
            ones8 = gpack[0:E, 42:50]      # [8, 8]
            sel_r = consts.tile([E, E * 128], f32r, tag="sel_r")
            eb_r = consts.tile([E, O], f32r, tag="eb_r")

            xTh = []
            for h2 in range(2):
                xt_t = stage.tile([128, (NI // 2) * BS], f32, tag=f"xT{h2}")
                xTh.append(xt_t)
            condT = stage.tile([C, BS], f32, tag="condT")
            gbs = []
            for e in range(E):
                gb_t = stage.tile([128, BS], f32, tag=f"gb{e}")
                gbs.append(gb_t)
            gT_r = stage.tile([E, BS], f32r, tag="gT_r")

            with (
                tc.tile_pool(name="ps_g", bufs=2, space="PSUM") as ps_g,
                tc.tile_pool(name="ps_junk", bufs=1, space="PSUM") as ps_junk,
            ):
                # HAM warm-up: keep the PE busy from engine boot so the
                # clock gate is at 8/8 when real matmuls arrive
                junk = stage.tile([128, 512], dt.bfloat16, tag="junk")
                nc.vector.memset(junk[:], 1.0)
                pj = ps_junk.tile([128, 512], f32, tag="junk")
                for i in range(10):
                    nc.tensor.matmul(pj[:], junk[:, 0:128], junk[:],
                                     start=(i == 0), stop=(i == 9))

                # ---- pre-transposed cond / x straight into SBUF ----
                nc.sync.dma_start(condT[:], condT_d)
                # xT halves as separate tiles so early matmuls only wait
                # for the first half: xTh[h][p, icl*BS + b]
                xs3 = xT_d.rearrange("(ic p) b -> p ic b", p=128)
                for h2 in range(2):
                    nc.sync.dma_start(
                        xTh[h2][:].rearrange("p (ic b) -> p ic b", ic=NI // 2),
                        xs3[:, h2 * (NI // 2):(h2 + 1) * (NI // 2), :])

                nc.gpsimd.dma_start(sel_r[:], sel_d)

                # ---- gating ----
                ph = ps_g.tile([128, BS], f32, tag="ps_g")
                nc.tensor.matmul(ph[0:H, :], gw1, condT[:],
                                 start=True, stop=True)
                hT = stage.tile([H, BS], f32, tag="hT")
                nc.vector.tensor_scalar(hT[:], ph[0:H, :], gb1, 0.0,
                                        mybir.AluOpType.add, mybir.AluOpType.max)
                pz = ps_g.tile([128, BS], f32, tag="ps_g")
                nc.tensor.matmul(pz[0:E, :], gw2, hT[:],
                                 start=True, stop=True)
                ezT = stage.tile([E, BS], f32, tag="ezT")
                nc.scalar.activation(ezT[:], pz[0:E, :],
                                     mybir.ActivationFunctionType.Exp,
                                     bias=gb2, scale=1.0)
                pden = ps_g.tile([128, BS], f32, tag="ps_g")
                nc.tensor.matmul(pden[0:E, :], ones8, ezT[:],
                                 start=True, stop=True)
                rden = stage.tile([E, BS], f32, tag="rden")
                nc.vector.reciprocal(rden[:], pden[0:E, :])
                # normalized gates, rounded to f32r (feeds bias + gb matmuls)
                nc.vector.tensor_mul(gT_r[:], ezT[:], rden[:])

                # gate rows broadcast to 128 partitions (fp32)
                for e in range(E):
                    pgb = ps_g.tile([128, BS], f32, tag="ps_g")
                    nc.tensor.matmul(pgb[:], sel_r[:, e * 128:(e + 1) * 128],
                                     gT_r[:], start=True, stop=True)
                    nc.vector.tensor_copy(gbs[e][:], pgb[:])

            # ---- main GEMM: 4 persistent [128,1024] PSUM accumulators ----
            with tc.tile_pool(name="ps_main", bufs=1, space="PSUM") as ps_main:
                pouts = []
                for bc in range(NB):
                    po = ps_main.tile([128, O], f32, tag=f"po{bc}")
                    pouts.append(po)
                for e in range(E):
                    wt = wpool.tile([128, NI * O], f32r, tag="w")
                    # wt[p, ic*O + o] = W[e*I + ic*128 + p, o]; casting DMAs
                    # split for queue parallelism (quarters for the boot-
                    # critical first expert)
                    nsp = 4 if e == 0 else 2
                    for h2 in range(nsp):
                        icn = NI // nsp
                        rows = w_d[e * I + h2 * icn * 128:
                                   e * I + (h2 + 1) * icn * 128, :]
                        nc.gpsimd.dma_start(
                            wt[:, h2 * icn * O:(h2 + 1) * icn * O]
                            .rearrange("p (ic o) -> p ic o", ic=icn),
                            rows.rearrange("(ic p) o -> p ic o", p=128))
                    if e == 0:
                        nc.gpsimd.dma_start(eb_r[:], eb_d)
                    # xtg_e = xT * g_e  (fp32 inputs, f32r output);
                    # one tile per ic so each matmul group waits only its own
                    xtgs = []
                    for ic in range(NI):
                        xtg_t = xtgp.tile([128, BS], f32r, tag=f"xtg{ic}")
                        xtgs.append(xtg_t)
                        nc.vector.tensor_mul(
                            xtg_t[:],
                            xTh[ic // (NI // 2)][:, (ic % (NI // 2)) * BS:
                                                 (ic % (NI // 2) + 1) * BS],
                            gbs[e][:])
                    if e < E - 1:
                        for ic in range(NI):
                            for bc in range(NB):
                                lhsT = xtgs[ic][:, bc * 128:(bc + 1) * 128]
                                for oh in range(NO2):
                                    nc.tensor.matmul(
                                        pouts[bc][:, oh * 512:(oh + 1) * 512],
                                        lhsT,
                                        wt[:, ic * O + oh * 512:
                                           ic * O + (oh + 1) * 512],
                                        start=(e == 0 and ic == 0), stop=False)
                    else:
                        # last expert bc-major: finish each batch chunk (bias
                        # + copy + store) while the others still compute
                        for bc in range(NB):
                            for ic in range(NI):
                                lhsT = xtgs[ic][:, bc * 128:(bc + 1) * 128]
                                for oh in range(NO2):
                                    nc.tensor.matmul(
                                        pouts[bc][:, oh * 512:(oh + 1) * 512],
                                        lhsT,
                                        wt[:, ic * O + oh * 512:
                                           ic * O + (oh + 1) * 512],
                                        start=False, stop=False)
                            for oh in range(NO2):
                                nc.tensor.matmul(
                                    pouts[bc][:, oh * 512:(oh + 1) * 512],
                                    gT_r[:, bc * 128:(bc + 1) * 128],
                                    eb_r[:, oh * 512:(oh + 1) * 512],
                                    start=False, stop=True)
                            osb = outp.tile([128, O], f32, tag="osb")
                            nc.vector.tensor_copy(osb[:], pouts[bc][:])
                            nc.sync.dma_start(
                                out_d[bc * 128:(bc + 1) * 128, :], osb[:])

    _split_waits(nc)
    return nc


def _get_nc():
    if "nc" not in _cache:
        _cache["nc"] = _build_nc()
    return _cache["nc"]


def _make_in_maps(x, cond, expert_weights, expert_biases, g_w1, g_b1, g_w2, g_b2):
    w_flat = np.ascontiguousarray(
        np.asarray(expert_weights, dtype=np.float32).reshape(E * I, O))
    xT = np.asarray(x, dtype=np.float32).T    # [I, B]
    condT = np.asarray(cond, dtype=np.float32).T  # [C, B]
    sel = np.zeros((E, E * 128), dtype=np.float32)
    for e in range(E):
        sel[e, e * 128:(e + 1) * 128] = 1.0
    gpack = np.zeros((128, 50), dtype=np.float32)
    gpack[:, 0:H] = np.asarray(g_w1, dtype=np.float32)
    gpack[0:H, H] = np.asarray(g_b1, dtype=np.float32)
    gpack[0:H, 33:33 + E] = np.asarray(g_w2, dtype=np.float32)
    gpack[0:E, 41] = np.asarray(g_b2, dtype=np.float32)
    gpack[0:E, 42:50] = 1.0
    common = {
        "w": w_flat,
        "eb": np.ascontiguousarray(np.asarray(expert_biases, dtype=np.float32)),
        "gpack": gpack,
        "sel": sel,
    }
    in_maps = []
    for c in range(N_CORES):
        m = dict(common)
        m["xT_sh"] = np.ascontiguousarray(xT[:, c * BS:(c + 1) * BS])
        m["condT_sh"] = np.ascontiguousarray(condT[:, c * BS:(c + 1) * BS])
        in_maps.append(m)
    return in_maps


def run(inputs, trace=False, **kw):
    """Build + run; returns (full_out [B, O] fp32, BassKernelResults)."""
    nc = _get_nc()
    in_maps = _make_in_maps(**inputs)
    res = run_bass_kernel_spmd(nc, in_maps, core_ids=list(range(N_CORES)),
                               trace=trace, **kw)
    out = np.concatenate([res.results[c]["out_sh"] for c in range(N_CORES)],
                         axis=0)
    return out, res


def kernel(**inputs):
    out, _ = run(inputs)
    return out



# revision 6
# speedup vs baseline: 1.0257x; 1.0257x over previous
"""MoE HyperNet linear layer on 8 Trainium2 NeuronCores.

Reference computation (B=4096, I=O=1024, C=128, E=8):
    h      = relu(cond @ g_w1 + g_b1)                # [B, 4E]
    gating = softmax(h @ g_w2 + g_b2, axis=1)        # [B, E]
    out    = einsum('be,beo->bo', gating,
                    einsum('bi,eio->beo', x, W)) + gating @ expert_biases

Strategy: data-parallel shard B across the 8 cores (512 rows each),
replicate all weights, and fold the (unnormalized) gate into the
activations:

    out[b,o] = (1/den[b]) * [ sum_e sum_i (ez[b,e]*x[b,i]) W_e[i,o]
                              + sum_e ez[b,e] eb[e,o] ]

so the whole MoE collapses into ONE K=8192 GEMM per core accumulated in
PSUM, with the softmax denominator folded into the final PSUM->SBUF
copy (a per-partition tensor_scalar multiply) instead of the gating
critical path.

Per core:
  - x/cond shards are passed pre-transposed ([feature, batch]); W is
    passed as bf16 [E*I, O] (host cast) so the weight stream is 16 MiB
    not 32 and all big-GEMM operands are bf16 (1 PE cycle/row).
  - gating MLP runs transposed in bf16 ([4E,512] -> [8,512]); exp on the
    Scalar engine (table prewarmed at boot), relu on DVE. No softmax
    normalization here: ez (unnormalized) is broadcast to 128 partitions
    with one-hot selector matmuls into PSUM scratch banks, copied to
    SBUF by GpSimd, and folded into xtg = xT * ez_b (DVE, bf16 out).
  - den[b] = sum_e ez[b,e] via 4 tiny matmuls (ezT chunk x ones column)
    into [128,1] PSUM columns -- batch-partition orientation -- then
    reciprocal_approx_fast; the final output copy is tensor_scalar_mul
    by rden (so normalization costs zero extra passes).
  - main GEMM: out[bc][b,o] += xtg_e[ic,bc].T @ W_e[ic,oh] accumulated
    over all (e, ic) in 4 persistent [128,1024] PSUM tiles (8 banks).
    The gating/broadcast/den temporaries live in those same banks as
    pre-GEMM scratch (tile deps order scratch readers before the
    chain-start matmuls).
  - expert 0's chain-start matmuls are staggered (all oh=0 chains
    through ic0..7 first, then oh=1 chains) so starts on banks 4-7
    never wait on the later gate-broadcast copies.
  - expert 7 runs bc-major with the bias matmul + oh-split scaled
    copy + store pipelined against the remaining chunks.
  - junk warmup matmuls are interleaved with the gating chain so the PE
    clock is ramped and the engine never idles from boot to the last
    main matmul.

Any instruction here can carry only ONE sync wait (walrus limit), so a
post-pass splits extra waits onto same-engine NoOps (_split_waits).
"""

import sys

if "/opt/trn_rl_repo" not in sys.path:
    sys.path.insert(0, "/opt/trn_rl_repo")

import ml_dtypes
import numpy as np

import bass_rust
import concourse.bass as bass
import concourse.mybir as mybir
import concourse.tile as tile
from concourse.bass_utils import run_bass_kernel_spmd


def _split_waits(nc, max_waits=1):
    """Hoist all-but-one sync wait of each instruction onto same-engine
    NoOps inserted directly before it. This walrus build rejects any TPB
    instruction carrying more than one wait ("Too many sync wait
    commands"); engines are in-order so the split preserves semantics."""
    for bb in nc.m.functions[0].blocks:
        out = []
        for i in list(bb.instructions):
            si = i.sync_info
            waits = list(si.on_wait) if si else []
            if len(waits) > max_waits:
                for k, w in enumerate(waits[:-max_waits]):
                    nop = mybir.InstNoOp(
                        name=f"{i.name}-waitsplit{k}", ins=[], outs=[])
                    nop.engine = i.engine
                    nop.sync_info = bass_rust.SyncInfo(on_wait=[w], on_update=[])
                    out.append(nop)
                i.sync_info = bass_rust.SyncInfo(
                    on_wait=waits[-max_waits:], on_update=list(si.on_update))
            out.append(i)
        bb.instructions = out

B, I, O, C, E = 4096, 1024, 1024, 128, 8
N_CORES = 8
BS = B // N_CORES          # 512 batch rows per core
NB = BS // 128             # 4 batch chunks of 128
NI = I // 128              # 8 contraction chunks
NO2 = 2                    # two N=512 halves of O
H = 4 * E                  # 32 gating hidden
GPB = 48 + E * 128         # bf16 gating pack width (gw1|gw2|ones|pad|sel)

_cache = {}


def _build_nc():
    dt = mybir.dt
    f32, bf16 = dt.float32, dt.bfloat16

    nc = bass.Bass("TRN2", target_bir_lowering=False, debug=False,
                   num_devices=N_CORES)

    xT_d = nc.dram_tensor("xT_sh", [I, BS], f32, kind="ExternalInput").ap()
    condT_d = nc.dram_tensor("condT_sh", [C, BS], bf16, kind="ExternalInput").ap()
    w_d = nc.dram_tensor("w", [E * I, O], bf16, kind="ExternalInput").ap()
    eb_d = nc.dram_tensor("eb", [E, O], bf16, kind="ExternalInput").ap()
    gpackb_d = nc.dram_tensor("gpackb", [128, GPB], bf16, kind="ExternalInput").ap()
    gpackf_d = nc.dram_tensor("gpackf", [128, 2], f32, kind="ExternalInput").ap()
    out_d = nc.dram_tensor("out_sh", [BS, O], f32, kind="ExternalOutput").ap()

    with tile.TileContext(nc) as tc:
        with (
            tc.tile_pool(name="consts", bufs=1) as consts,
            tc.tile_pool(name="stage", bufs=1) as stage,
            tc.tile_pool(name="wpool", bufs=3) as wpool,
            tc.tile_pool(name="xtgp", bufs=3) as xtgp,
            tc.tile_pool(name="outp", bufs=2) as outp,
        ):
            # ---- SBUF tiles ----
            condT = stage.tile([C, BS], bf16, tag="condT")
            gpackb = consts.tile([128, GPB], bf16, tag="gpackb")
            gpackf = consts.tile([128, 2], f32, tag="gpackf")
            warm = consts.tile([1, 8], f32, tag="warm")
            warm2 = consts.tile([1, 8], f32, tag="warm2")
            junk = consts.tile([128, 512], bf16, tag="junk")
            eb_t = consts.tile([E, O], bf16, tag="eb")
            xTh = []
            for h2 in range(2):
                xt_t = stage.tile([128, (NI // 2) * BS], f32, tag=f"xT{h2}")
                xTh.append(xt_t)
            hT = stage.tile([H, BS], bf16, tag="hT")
            ezT = stage.tile([E, BS], bf16, tag="ezT")
            rden = stage.tile([128, NB], f32, tag="rden")
            gbs = []
            for e in range(E):
                gb_t = stage.tile([128, BS], f32, tag=f"gb{e}")
                gbs.append(gb_t)

            gw1 = gpackb[:, 0:H]               # [128, 32]
            gw2 = gpackb[0:H, H:H + E]         # [32, 8]
            onescol = gpackb[0:E, 40:41]       # [8, 1]
            gb1 = gpackf[0:H, 0:1]             # [32, 1]
            gb2 = gpackf[0:E, 1:2]             # [8, 1]

            with tc.tile_pool(name="ps_main", bufs=1, space="PSUM") as ps_main:
                pouts = []
                for bc in range(NB):
                    po = ps_main.tile([128, O], f32, tag=f"po{bc}")
                    pouts.append(po)
                # PSUM scratch aliases (pre-GEMM): gating temporaries and the
                # 8 gate-broadcast targets live in the accumulator banks.
                ph = pouts[0][0:H, 0:512]      # bank 0
                pz = pouts[1][0:E, 0:512]      # bank 2
                pden = pouts[2][:, 0:NB]       # bank 4, [128, 4]

                def pgb(e):
                    return pouts[e % NB][:, (e // NB) * 512:(e // NB) * 512 + 512]

                def junk_mm():
                    nc.tensor.matmul(pouts[3][:, 512:1024], junk[:, 0:128],
                                     junk[:], start=True, stop=True)

                # ---- DMA programs (queue order matters) ----
                # sync queue: cond, gating packs, x chunks
                nc.sync.dma_start(condT[:], condT_d)
                nc.sync.dma_start(gpackf[:], gpackf_d)
                nc.sync.dma_start(gpackb[:], gpackb_d)
                xs3 = xT_d.rearrange("(ic p) b -> p ic b", p=128)
                for ic in range(NI):
                    nc.sync.dma_start(
                        xTh[ic // (NI // 2)][:, (ic % (NI // 2)) * BS:
                                             (ic % (NI // 2) + 1) * BS],
                        xs3[:, ic, :])

                # scalar queue: exp-table prewarm (activation loads its table
                # once here, not on the gating critical path)
                nc.scalar.memzero(warm[:])
                nc.scalar.activation(warm2[:], warm[:],
                                     mybir.ActivationFunctionType.Exp,
                                     bias=0.0, scale=1.0)

                # vector queue head: junk init, then (parked) relu
                nc.vector.memset(junk[:], 0.5)

                # gpsimd queue: W expert 0 in quarters, eb, then gate copies
                wt0 = wpool.tile([128, NI * O], bf16, tag="w")
                for q in range(4):
                    icn = NI // 4
                    rows = w_d[q * icn * 128:(q + 1) * icn * 128, :]
                    nc.gpsimd.dma_start(
                        wt0[:, q * icn * O:(q + 1) * icn * O]
                        .rearrange("p (ic o) -> p ic o", ic=icn),
                        rows.rearrange("(ic p) o -> p ic o", p=128))
                nc.gpsimd.dma_start(eb_t[:], eb_d)

                # ---- gating chain (PE interleaved with junk warmup) ----
                junk_mm()
                junk_mm()
                nc.tensor.matmul(ph, gw1, condT[:], start=True, stop=True)
                # relu on DVE (parked, waits ph)
                nc.vector.tensor_scalar(hT[:], ph, gb1, 0.0,
                                        mybir.AluOpType.add,
                                        mybir.AluOpType.max)
                junk_mm()
                junk_mm()
                nc.tensor.matmul(pz, gw2, hT[:], start=True, stop=True)
                # exp on Scalar (table already warm), bf16 out
                nc.scalar.activation(ezT[:], pz,
                                     mybir.ActivationFunctionType.Exp,
                                     bias=gb2, scale=1.0)
                junk_mm()
                junk_mm()
                junk_mm()
                # den[b] = sum_e ez[b,e] in batch-partition orientation
                for bc in range(NB):
                    nc.tensor.matmul(pden[:, bc:bc + 1],
                                     ezT[:, bc * 128:(bc + 1) * 128],
                                     onescol, start=True, stop=True)
                # off the critical path (first consumer is the e7 output
                # copies), so the exact-but-slower reciprocal is fine here
                nc.vector.reciprocal(rden[:], pden)
                # broadcast unnormalized gates to 128 partitions
                for e in range(E):
                    nc.tensor.matmul(pgb(e),
                                     gpackb[0:E, 48 + e * 128:48 + (e + 1) * 128],
                                     ezT[:], start=True, stop=True)
                # copies for the oh=0 scratch banks (GpSimd can't read PSUM,
                # so these ride the DVE queue ahead of the e0 xtg batch)
                for e in range(NB):
                    nc.vector.tensor_copy(gbs[e][:], pgb(e))

                # ---- main GEMM ----
                for e in range(E):
                    if e > 0:
                        wt = wpool.tile([128, NI * O], bf16, tag="w")
                        for h2 in range(2):
                            icn = NI // 2
                            rows = w_d[e * I + h2 * icn * 128:
                                       e * I + (h2 + 1) * icn * 128, :]
                            nc.gpsimd.dma_start(
                                wt[:, h2 * icn * O:(h2 + 1) * icn * O]
                                .rearrange("p (ic o) -> p ic o", ic=icn),
                                rows.rearrange("(ic p) o -> p ic o", p=128))
                    else:
                        wt = wt0
                    # xtg_e = xT * ez_e  (fp32 in, bf16 out)
                    xtgs = []
                    for ic in range(NI):
                        xtg_t = xtgp.tile([128, BS], bf16, tag=f"xtg{ic}")
                        xtgs.append(xtg_t)
                        nc.vector.tensor_mul(
                            xtg_t[:],
                            xTh[ic // (NI // 2)][:, (ic % (NI // 2)) * BS:
                                                 (ic % (NI // 2) + 1) * BS],
                            gbs[e][:])
                    if e == 0:
                        # copies for the oh=1 scratch banks — needed only by
                        # the staggered oh=1 chain starts, so they follow the
                        # e0 xtg batch on the DVE queue
                        for e2 in range(NB, E):
                            nc.vector.tensor_copy(gbs[e2][:], pgb(e2))
                    if e == 0:
                        # staggered starts: all oh=0 chains first, then oh=1,
                        # so chain-start matmuls on banks 4-7 never wait on
                        # the later gate-broadcast copies.
                        for oh in range(NO2):
                            for ic in range(NI):
                                for bc in range(NB):
                                    nc.tensor.matmul(
                                        pouts[bc][:, oh * 512:(oh + 1) * 512],
                                        xtgs[ic][:, bc * 128:(bc + 1) * 128],
                                        wt[:, ic * O + oh * 512:
                                           ic * O + (oh + 1) * 512],
                                        start=(ic == 0), stop=False)
                    elif e < E - 1:
                        for ic in range(NI):
                            for bc in range(NB):
                                lhsT = xtgs[ic][:, bc * 128:(bc + 1) * 128]
                                for oh in range(NO2):
                                    nc.tensor.matmul(
                                        pouts[bc][:, oh * 512:(oh + 1) * 512],
                                        lhsT,
                                        wt[:, ic * O + oh * 512:
                                           ic * O + (oh + 1) * 512],
                                        start=False, stop=False)
                    else:
                        # last expert bc-major: finish each batch chunk (bias
                        # + scaled copy + store) while the others compute
                        for bc in range(NB):
                            osb = outp.tile([128, O], f32, tag="osb")
                            for oh in range(NO2):
                                for ic in range(NI):
                                    nc.tensor.matmul(
                                        pouts[bc][:, oh * 512:(oh + 1) * 512],
                                        xtgs[ic][:, bc * 128:(bc + 1) * 128],
                                        wt[:, ic * O + oh * 512:
                                           ic * O + (oh + 1) * 512],
                                        start=False, stop=False)
                                # expert-bias term, then close the chain
                                nc.tensor.matmul(
                                    pouts[bc][:, oh * 512:(oh + 1) * 512],
                                    ezT[:, bc * 128:(bc + 1) * 128],
                                    eb_t[:, oh * 512:(oh + 1) * 512],
                                    start=False, stop=True)
                                # softmax normalization folded into the copy
                                nc.vector.tensor_scalar_mul(
                                    osb[:, oh * 512:(oh + 1) * 512],
                                    pouts[bc][:, oh * 512:(oh + 1) * 512],
                                    rden[:, bc:bc + 1])
                            nc.sync.dma_start(
                                out_d[bc * 128:(bc + 1) * 128, :], osb[:])

    _split_waits(nc)
    return nc


def _get_nc():
    if "nc" not in _cache:
        _cache["nc"] = _build_nc()
    return _cache["nc"]


def _make_in_maps(x, cond, expert_weights, expert_biases, g_w1, g_b1, g_w2, g_b2):
    bf16 = ml_dtypes.bfloat16
    w_flat = np.ascontiguousarray(
        np.asarray(expert_weights, dtype=np.float32).reshape(E * I, O)
    ).astype(bf16)
    xT = np.asarray(x, dtype=np.float32).T              # [I, B]
    condT = np.asarray(cond, dtype=np.float32).T.astype(bf16)  # [C, B]
    gpackb = np.zeros((128, GPB), dtype=bf16)
    gpackb[:, 0:H] = np.asarray(g_w1, dtype=np.float32).astype(bf16)
    gpackb[0:H, H:H + E] = np.asarray(g_w2, dtype=np.float32).astype(bf16)
    gpackb[0:E, 40] = 1.0
    for e in range(E):
        gpackb[e, 48 + e * 128:48 + (e + 1) * 128] = 1.0
    gpackf = np.zeros((128, 2), dtype=np.float32)
    gpackf[0:H, 0] = np.asarray(g_b1, dtype=np.float32)
    gpackf[0:E, 1] = np.asarray(g_b2, dtype=np.float32)
    common = {
        "w": w_flat,
        "eb": np.ascontiguousarray(
            np.asarray(expert_biases, dtype=np.float32)).astype(bf16),
        "gpackb": gpackb,
        "gpackf": gpackf,
    }
    in_maps = []
    for c in range(N_CORES):
        m = dict(common)
        m["xT_sh"] = np.ascontiguousarray(xT[:, c * BS:(c + 1) * BS])
        m["condT_sh"] = np.ascontiguousarray(condT[:, c * BS:(c + 1) * BS])
        in_maps.append(m)
    return in_maps


def run(inputs, trace=False, **kw):
    """Build + run; returns (full_out [B, O] fp32, BassKernelResults)."""
    nc = _get_nc()
    in_maps = _make_in_maps(**inputs)
    res = run_bass_kernel_spmd(nc, in_maps, core_ids=list(range(N_CORES)),
                               trace=trace, **kw)
    out = np.concatenate([res.results[c]["out_sh"] for c in range(N_CORES)],
                         axis=0)
    return out, res


def kernel(**inputs):
    out, _ = run(inputs)
    return out


# revision 8
# speedup vs baseline: 1.1203x; 1.0922x over previous
"""MoE HyperNet linear layer on 8 Trainium2 NeuronCores.

Reference computation (B=4096, I=O=1024, C=128, E=8):
    h      = relu(cond @ g_w1 + g_b1)                # [B, 4E]
    gating = softmax(h @ g_w2 + g_b2, axis=1)        # [B, E]
    out    = einsum('be,beo->bo', gating,
                    einsum('bi,eio->beo', x, W)) + gating @ expert_biases

Strategy: data-parallel shard B across the 8 cores (512 rows each),
replicate all weights, and fold the (unnormalized) gate into the
activations:

    out[b,o] = (1/den[b]) * [ sum_e sum_i (ez[b,e]*x[b,i]) W_e[i,o]
                              + sum_e ez[b,e] eb[e,o] ]

so the whole MoE collapses into ONE K=8192 GEMM per core accumulated in
PSUM, with the softmax denominator folded into the final PSUM->SBUF
copy (a per-partition tensor_scalar multiply) off the gating critical
path.

Layout per core:
  - All big-GEMM operands are bf16 (1 PE cycle/row): x/cond arrive
    pre-transposed and host-cast to bf16; W arrives as bf16 [E*I, O]
    (16 MiB weight stream) and stays RESIDENT in SBUF (8 x 16KB/part).
  - The O dimension is processed in two 512-wide passes, so the PSUM
    accumulators are 4 x [128,512] (banks 0-3) and the gating scratch
    (ph/pz/pden + gate broadcasts) gets its own 4 banks -- no PSUM
    aliasing against accumulator chains, clean tile deps.
  - gating MLP transposed in bf16; exp on Scalar (table prewarmed at
    boot); relu/reciprocal on DVE. den[b] = sum_e ez[b,e] via 4 tiny
    matmuls in batch-partition orientation ([8,128] ezT chunk x ones
    column), so normalization never touches the gating critical path.
  - gate broadcast e: one-hot selector matmul into a scratch bank,
    DVE-copied to SBUF as bf16 (exact: ez is already bf16). bcast_{e+1}
    is emitted just before expert e's matmul block, so the PE pays
    ~0.2us per expert with banks reused at ~28us spacing.
  - xtg_e = xT * ez_b (DVE, bf16) is recomputed per O-pass; DVE is
    half-loaded so this costs no wall-clock. Pass-2's first xtg batch is
    emitted before pass-1's output drain to hide the pass boundary.
  - expert 7 of each pass runs bc-major: ic matmuls + expert-bias
    matmul (stop) + rden-scaled copy + store per 128-row chunk, so the
    drain pipelines against the remaining chunks.
  - junk warmup matmuls fill the PE from boot through the gating chain
    so the clock is ramped when the main GEMM starts.

Any instruction here can carry only ONE sync wait (walrus limit), so a
post-pass splits extra waits onto same-engine NoOps (_split_waits).
"""

import sys

if "/opt/trn_rl_repo" not in sys.path:
    sys.path.insert(0, "/opt/trn_rl_repo")

import ml_dtypes
import numpy as np

import bass_rust
import concourse.bass as bass
import concourse.mybir as mybir
import concourse.tile as tile
from concourse.bass_utils import run_bass_kernel_spmd


def _split_waits(nc, max_waits=1):
    """Hoist all-but-one sync wait of each instruction onto same-engine
    NoOps inserted directly before it. This walrus build rejects any TPB
    instruction carrying more than one wait ("Too many sync wait
    commands"); engines are in-order so the split preserves semantics."""
    for bb in nc.m.functions[0].blocks:
        out = []
        for i in list(bb.instructions):
            si = i.sync_info
            waits = list(si.on_wait) if si else []
            if len(waits) > max_waits:
                for k, w in enumerate(waits[:-max_waits]):
                    nop = mybir.InstNoOp(
                        name=f"{i.name}-waitsplit{k}", ins=[], outs=[])
                    nop.engine = i.engine
                    nop.sync_info = bass_rust.SyncInfo(on_wait=[w], on_update=[])
                    out.append(nop)
                i.sync_info = bass_rust.SyncInfo(
                    on_wait=waits[-max_waits:], on_update=list(si.on_update))
            out.append(i)
        bb.instructions = out

B, I, O, C, E = 4096, 1024, 1024, 128, 8
N_CORES = 8
BS = B // N_CORES          # 512 batch rows per core
NB = BS // 128             # 4 batch chunks of 128
NI = I // 128              # 8 contraction chunks
NO2 = 2                    # two N=512 passes over O
H = 4 * E                  # 32 gating hidden
GPB = 48 + E * 128         # bf16 gating pack width (gw1|gw2|ones|pad|sel)

_cache = {}


def _build_nc():
    dt = mybir.dt
    f32, bf16 = dt.float32, dt.bfloat16

    nc = bass.Bass("TRN2", target_bir_lowering=False, debug=False,
                   num_devices=N_CORES)

    xT_d = nc.dram_tensor("xT_sh", [I, BS], bf16, kind="ExternalInput").ap()
    condT_d = nc.dram_tensor("condT_sh", [C, BS], bf16, kind="ExternalInput").ap()
    w_d = nc.dram_tensor("w", [E * I, O], bf16, kind="ExternalInput").ap()
    eb_d = nc.dram_tensor("eb", [E, O], bf16, kind="ExternalInput").ap()
    gpackb_d = nc.dram_tensor("gpackb", [128, GPB], bf16, kind="ExternalInput").ap()
    gpackf_d = nc.dram_tensor("gpackf", [128, 2], f32, kind="ExternalInput").ap()
    out_d = nc.dram_tensor("out_sh", [BS, O], f32, kind="ExternalOutput").ap()

    with tile.TileContext(nc) as tc:
        with (
            tc.tile_pool(name="consts", bufs=1) as consts,
            tc.tile_pool(name="stage", bufs=1) as stage,
            tc.tile_pool(name="xtgp", bufs=2) as xtgp,
            tc.tile_pool(name="outp", bufs=2) as outp,
        ):
            # ---- SBUF tiles ----
            condT = stage.tile([C, BS], bf16, tag="condT")
            gpackb = consts.tile([128, GPB], bf16, tag="gpackb")
            gpackf = consts.tile([128, 2], f32, tag="gpackf")
            warm = consts.tile([1, 8], f32, tag="warm")
            warm2 = consts.tile([1, 8], f32, tag="warm2")
            junk = consts.tile([128, 512], bf16, tag="junk")
            eb_t = consts.tile([E, O], bf16, tag="eb")
            xT = stage.tile([128, NI * BS], bf16, tag="xT")
            hT = stage.tile([H, BS], bf16, tag="hT")
            ezT = stage.tile([E, BS], bf16, tag="ezT")
            rden = stage.tile([128, NB], f32, tag="rden")
            gbs = []
            for e in range(E):
                gb_t = stage.tile([128, BS], bf16, tag=f"gb{e}")
                gbs.append(gb_t)
            wts = []
            for e in range(E):
                wt_t = consts.tile([128, NI * O], bf16, tag=f"w{e}")
                wts.append(wt_t)

            gw1 = gpackb[:, 0:H]               # [128, 32]
            gw2 = gpackb[0:H, H:H + E]         # [32, 8]
            onescol = gpackb[0:E, 40:41]       # [8, 1]
            gb1 = gpackf[0:H, 0:1]             # [32, 1]
            gb2 = gpackf[0:E, 1:2]             # [8, 1]

            with (
                tc.tile_pool(name="ps_main", bufs=1, space="PSUM") as ps_main,
                tc.tile_pool(name="ps_g", bufs=1, space="PSUM") as ps_g,
            ):
                pouts = []
                for bc in range(NB):
                    po = ps_main.tile([128, 512], f32, tag=f"po{bc}")
                    pouts.append(po)
                gtiles = []
                for k in range(4):
                    gt = ps_g.tile([128, 512], f32, tag=f"g{k}")
                    gtiles.append(gt)
                ph = gtiles[0][0:H, :]
                pz = gtiles[1][0:E, :]
                pden = gtiles[2][:, 0:NB]

                def junk_mm():
                    nc.tensor.matmul(gtiles[3][:, :], junk[:, 0:128],
                                     junk[:], start=True, stop=True)

                # ---- DMA programs (queue order matters) ----
                # scalar queue: the small gating inputs (idle DMA queue, so
                # they land early), then the exp-table prewarm
                nc.scalar.dma_start(gpackf[:], gpackf_d)
                nc.scalar.dma_start(gpackb[:], gpackb_d)
                nc.scalar.dma_start(condT[:], condT_d)
                nc.scalar.memzero(warm[:])
                nc.scalar.activation(warm2[:], warm[:],
                                     mybir.ActivationFunctionType.Exp,
                                     bias=0.0, scale=1.0)

                # sync queue: x chunks (bf16, 128 KiB each)
                xs3 = xT_d.rearrange("(ic p) b -> p ic b", p=128)
                for ic in range(NI):
                    nc.sync.dma_start(xT[:, ic * BS:(ic + 1) * BS],
                                      xs3[:, ic, :])

                # gpsimd queue: all 8 resident W tiles (e0 split in quarters
                # for an early first chunk), eb
                for e in range(E):
                    nsp = 4 if e == 0 else 2
                    for h2 in range(nsp):
                        icn = NI // nsp
                        rows = w_d[e * I + h2 * icn * 128:
                                   e * I + (h2 + 1) * icn * 128, :]
                        nc.gpsimd.dma_start(
                            wts[e][:, h2 * icn * O:(h2 + 1) * icn * O]
                            .rearrange("p (ic o) -> p ic o", ic=icn),
                            rows.rearrange("(ic p) o -> p ic o", p=128))
                    if e == 0:
                        nc.gpsimd.dma_start(eb_t[:], eb_d)

                # vector queue head
                nc.vector.memset(junk[:], 0.5)

                # ---- gating chain (PE interleaved with junk warmup) ----
                junk_mm()
                junk_mm()
                junk_mm()
                nc.tensor.matmul(ph, gw1, condT[:], start=True, stop=True)
                nc.vector.tensor_scalar(hT[:], ph, gb1, 0.0,
                                        mybir.AluOpType.add,
                                        mybir.AluOpType.max)
                junk_mm()
                nc.tensor.matmul(pz, gw2, hT[:], start=True, stop=True)
                nc.scalar.activation(ezT[:], pz,
                                     mybir.ActivationFunctionType.Exp,
                                     bias=gb2, scale=1.0)
                junk_mm()
                junk_mm()
                # den[b] = sum_e ez[b,e] in batch-partition orientation;
                # consumed (reciprocal) off the critical path
                for bc in range(NB):
                    nc.tensor.matmul(pden[:, bc:bc + 1],
                                     ezT[:, bc * 128:(bc + 1) * 128],
                                     onescol, start=True, stop=True)
                nc.vector.reciprocal(rden[:], pden)

                def bcast(e):
                    # one-hot selector matmul: gtiles[e%4][p, b] = ez[e, b]
                    nc.tensor.matmul(gtiles[e % 4][:, :],
                                     gpackb[0:E, 48 + e * 128:48 + (e + 1) * 128],
                                     ezT[:], start=True, stop=True)

                bcast(0)
                junk_mm()
                junk_mm()

                # ---- main GEMM: two 512-wide passes over O ----
                for oh in range(NO2):
                    for e in range(E):
                        if oh == 0:
                            if e < E - 1:
                                bcast(e + 1)  # PE fills ~0.2us, huge slack
                            nc.vector.tensor_copy(gbs[e][:], gtiles[e % 4][:, :])
                        # xtg_e = xT * ez_e  (bf16 in/out), recomputed per pass
                        xtgs = []
                        for ic in range(NI):
                            xtg_t = xtgp.tile([128, BS], bf16, tag=f"xtg{ic}")
                            xtgs.append(xtg_t)
                            nc.vector.tensor_mul(
                                xtg_t[:], xT[:, ic * BS:(ic + 1) * BS],
                                gbs[e][:])
                        if e < E - 1:
                            for ic in range(NI):
                                for bc in range(NB):
                                    nc.tensor.matmul(
                                        pouts[bc][:, :],
                                        xtgs[ic][:, bc * 128:(bc + 1) * 128],
                                        wts[e][:, ic * O + oh * 512:
                                               ic * O + oh * 512 + 512],
                                        start=(e == 0 and ic == 0), stop=False)
                        else:
                            # last expert bc-major: finish each batch chunk
                            # (bias + scaled copy + store) while the others
                            # still compute
                            for bc in range(NB):
                                for ic in range(NI):
                                    nc.tensor.matmul(
                                        pouts[bc][:, :],
                                        xtgs[ic][:, bc * 128:(bc + 1) * 128],
                                        wts[e][:, ic * O + oh * 512:
                                               ic * O + oh * 512 + 512],
                                        start=False, stop=False)
                                nc.tensor.matmul(
                                    pouts[bc][:, :],
                                    ezT[:, bc * 128:(bc + 1) * 128],
                                    eb_t[:, oh * 512:(oh + 1) * 512],
                                    start=False, stop=True)
                                # rden-scaled copy on the idle Scalar engine
                                # so the DVE queue stays pure xtg and pass
                                # 2's first batch runs ahead of this drain
                                osb = outp.tile([128, 512], f32, tag="osb")
                                nc.scalar.mul(osb[:], pouts[bc][:, :],
                                              rden[:, bc:bc + 1])
                                nc.sync.dma_start(
                                    out_d[bc * 128:(bc + 1) * 128,
                                          oh * 512:(oh + 1) * 512],
                                    osb[:])

    _split_waits(nc)
    return nc


def _get_nc():
    if "nc" not in _cache:
        _cache["nc"] = _build_nc()
    return _cache["nc"]


def _make_in_maps(x, cond, expert_weights, expert_biases, g_w1, g_b1, g_w2, g_b2):
    bf16 = ml_dtypes.bfloat16
    w_flat = np.ascontiguousarray(
        np.asarray(expert_weights, dtype=np.float32).reshape(E * I, O)
    ).astype(bf16)
    xT = np.asarray(x, dtype=np.float32).T.astype(bf16)        # [I, B]
    condT = np.asarray(cond, dtype=np.float32).T.astype(bf16)  # [C, B]
    gpackb = np.zeros((128, GPB), dtype=bf16)
    gpackb[:, 0:H] = np.asarray(g_w1, dtype=np.float32).astype(bf16)
    gpackb[0:H, H:H + E] = np.asarray(g_w2, dtype=np.float32).astype(bf16)
    gpackb[0:E, 40] = 1.0
    for e in range(E):
        gpackb[e, 48 + e * 128:48 + (e + 1) * 128] = 1.0
    gpackf = np.zeros((128, 2), dtype=np.float32)
    gpackf[0:H, 0] = np.asarray(g_b1, dtype=np.float32)
    gpackf[0:E, 1] = np.asarray(g_b2, dtype=np.float32)
    common = {
        "w": w_flat,
        "eb": np.ascontiguousarray(
            np.asarray(expert_biases, dtype=np.float32)).astype(bf16),
        "gpackb": gpackb,
        "gpackf": gpackf,
    }
    in_maps = []
    for c in range(N_CORES):
        m = dict(common)
        m["xT_sh"] = np.ascontiguousarray(xT[:, c * BS:(c + 1) * BS])
        m["condT_sh"] = np.ascontiguousarray(condT[:, c * BS:(c + 1) * BS])
        in_maps.append(m)
    return in_maps


def run(inputs, trace=False, **kw):
    """Build + run; returns (full_out [B, O] fp32, BassKernelResults)."""
    nc = _get_nc()
    in_maps = _make_in_maps(**inputs)
    res = run_bass_kernel_spmd(nc, in_maps, core_ids=list(range(N_CORES)),
                               trace=trace, **kw)
    out = np.concatenate([res.results[c]["out_sh"] for c in range(N_CORES)],
                         axis=0)
    return out, res


def kernel(**inputs):
    out, _ = run(inputs)
    return out
